# revision 4
# baseline (speedup 1.0000x reference)
"""Multi-head attention (B=2, S=2048, D=1024, H=16) on 8 Trainium2 cores.

Sharding: batch x head-group. Core c handles batch b = c // 4 and heads
[4*(c%4), 4*(c%4)+4). Each core computes its Q/K/V projection slices
(transposed layout), per-head attention with softmax, the attention-weight
output block, and a partial output projection over its 256 channels.
Host sums the 4 partial output projections per batch element and adds bo.

All device compute is fp32. Scores are built transposed ([j, i] = [key,
query]) so the PV matmul can consume them directly; a ones column appended
to V yields the softmax denominators from the same matmul. exp() normalizer
is applied with a PE-broadcast reciprocal row. The attention output is
written [h, j, i] and transposed on the host.
"""

import numpy as np

import concourse.bass as bass
import concourse.tile as tile
from concourse import bacc, mybir
from concourse.bass import ds
from concourse.bass_utils import run_bass_kernel_spmd

F32 = mybir.dt.float32
AF = mybir.ActivationFunctionType

B = 2
S = 2048
D = 1024
NH = 16
DK = 64
NCORES = 8
GPB = 4                # cores (head-groups) per batch element
HPC = NH // GPB        # heads per core = 4
DC = HPC * DK          # channels per core = 256
KC = D // 128          # contraction chunks = 8
NJT = S // 128         # j tiles = 16
IBW = 1024             # i block width
NIB = S // IBW         # i blocks = 2
NST = S // 128         # s tiles for V projection = 16
SCALE = float(1.0 / np.sqrt(np.float32(DK)))

_CACHE = {}


def build_nc():
    nc = bacc.Bacc("TRN2", target_bir_lowering=False, debug=False)

    xq = nc.declare_dram_parameter("xqT", [D, S], F32, isOutput=False)
    xk = nc.declare_dram_parameter("xkT", [D, S], F32, isOutput=False)
    xv = nc.declare_dram_parameter("xvT", [D, S], F32, isOutput=False)
    wq = nc.declare_dram_parameter("wq", [D, DC], F32, isOutput=False)
    wk = nc.declare_dram_parameter("wk", [D, DC], F32, isOutput=False)
    wv = nc.declare_dram_parameter("wv", [D, DC], F32, isOutput=False)
    wo = nc.declare_dram_parameter("wo", [DC, D], F32, isOutput=False)
    bq = nc.declare_dram_parameter("bq", [1, DC], F32, isOutput=False)
    bk = nc.declare_dram_parameter("bk", [1, DC], F32, isOutput=False)
    bv = nc.declare_dram_parameter("bv", [1, DC], F32, isOutput=False)
    attnT = nc.declare_dram_parameter("attnT", [HPC, S, S], F32, isOutput=True)
    outp = nc.declare_dram_parameter("outp", [S, D], F32, isOutput=True)

    # chunked DRAM views: row k = c*128 + p  ->  [p, c, ...]
    xq_c = xq.ap().rearrange("(c p) s -> p c s", p=128)
    xk_c = xk.ap().rearrange("(c p) s -> p c s", p=128)
    xv_c = xv.ap().rearrange("(c p) s -> p c s", p=128)
    wq_c = wq.ap().rearrange("(c p) n -> p c n", p=128)
    wk_c = wk.ap().rearrange("(c p) n -> p c n", p=128)
    wv_c = wv.ap().rearrange("(c p) n -> p c n", p=128)
    wo_c = wo.ap().rearrange("(c p) n -> p c n", p=128)
    attnT_ap = attnT.ap()
    outp_ap = outp.ap()

    with tile.TileContext(nc) as tc:
        with tc.tile_pool(name="persist", bufs=1) as pp:
            qT = pp.tile([128, 2, S], F32, tag="qT")     # [pair-chan, pair, s]
            kT = pp.tile([128, 2, S], F32, tag="kT")
            vsb = pp.tile([128, NST, HPC, DK + 1], F32, tag="vsb")
            wo_sb = pp.tile([128, 2, D], F32, tag="wo")
            outT = pp.tile([128, 2, S], F32, tag="outT")  # [pair-chan, pair, i]
            ones_k1 = pp.tile([1, 128], F32, tag="ones_k1")

            nc.any.memset(ones_k1[:], 1.0)
            nc.any.memset(vsb[:, :, :, DK], 1.0)  # denominator ones column
            nc.sync.dma_start(out=wo_sb[:], in_=wo_c)

            # ---------------- Phase 1: projections ----------------
            with tc.tile_pool(name="ph1", bufs=1) as wp:
                wq_sb = wp.tile([128, KC, DC], F32, tag="wq")
                wk_sb = wp.tile([128, KC, DC], F32, tag="wk")
                wv_sb = wp.tile([128, KC, DC], F32, tag="wv")
                bq_sb = wp.tile([1, DC], F32, tag="bq")
                bk_sb = wp.tile([1, DC], F32, tag="bk")
                bv_sb = wp.tile([1, DC], F32, tag="bv")
                ones_row = wp.tile([1, 512], F32, tag="ones_row")
                nc.sync.dma_start(out=wq_sb[:], in_=wq_c)
                nc.sync.dma_start(out=wk_sb[:], in_=wk_c)
                nc.sync.dma_start(out=wv_sb[:], in_=wv_c)
                nc.sync.dma_start(out=bq_sb[:], in_=bq.ap())
                nc.sync.dma_start(out=bk_sb[:], in_=bk.ap())
                nc.sync.dma_start(out=bv_sb[:], in_=bv.ap())
                nc.any.memset(ones_row[:], 1.0)

                # QT / KT: out[d, s] = sum_k W[k, d] * xT[k, s]  (+ bias)
                with tc.tile_pool(name="ph1qk", bufs=2, space="PSUM") as pqk, \
                     tc.tile_pool(name="ph1x", bufs=3) as px:
                    for (x_c, w_sb, b_sb, dst) in (
                        (xq_c, wq_sb, bq_sb, qT),
                        (xk_c, wk_sb, bk_sb, kT),
                    ):
                        ps = [
                            pqk.tile([128, S], F32, tag="pqk", name=f"pqk{i}")
                            for i in range(2)
                        ]
                        for kc in range(KC):
                            xc = px.tile([128, S], F32, tag="xc")
                            nc.sync.dma_start(out=xc[:], in_=x_c[:, kc])
                            for pair in range(2):
                                lhsT = w_sb[:, kc, ds(pair * 128, 128)]
                                for n in range(S // 512):
                                    nc.tensor.matmul(
                                        ps[pair][:, ds(n * 512, 512)],
                                        lhsT,
                                        xc[:, ds(n * 512, 512)],
                                        start=(kc == 0),
                                        stop=False,
                                    )
                        # bias row: out[d, s] += b[d] * 1
                        for pair in range(2):
                            lhsT_b = b_sb[:, ds(pair * 128, 128)]
                            for n in range(S // 512):
                                nc.tensor.matmul(
                                    ps[pair][:, ds(n * 512, 512)],
                                    lhsT_b,
                                    ones_row[:],
                                    start=False,
                                    stop=(n == S // 512 - 1),
                                )
                            nc.scalar.copy(dst[:, pair], ps[pair][:])

                # V: out[s, d] = sum_k xvT[k, s] * W[k, d]  (+ bias)
                with tc.tile_pool(name="ph1v", bufs=4, space="PSUM") as pv, \
                     tc.tile_pool(name="ph1xv", bufs=3) as pxv:
                    for stg in range(4):  # groups of 4 s-tiles
                        xvs = []
                        for kc in range(KC):
                            xt = pxv.tile([128, 512], F32, tag="xv")
                            nc.sync.dma_start(
                                out=xt[:], in_=xv_c[:, kc, ds(stg * 512, 512)]
                            )
                            xvs.append(xt)
                        for sti in range(4):
                            st = stg * 4 + sti
                            psv = pv.tile([128, DC], F32, tag="psv")
                            for kc in range(KC):
                                nc.tensor.matmul(
                                    psv[:],
                                    xvs[kc][:, ds(sti * 128, 128)],
                                    wv_sb[:, kc],
                                    start=(kc == 0),
                                    stop=False,
                                )
                            nc.tensor.matmul(
                                psv[:], ones_k1[:], bv_sb[:],
                                start=False, stop=True,
                            )
                            nc.scalar.copy(
                                vsb[:, st, :, 0:DK],
                                psv[:].rearrange("p (h d) -> p h d", h=HPC),
                            )

            # ---------------- Phase 2: attention ----------------
            with tc.tile_pool(name="psc", bufs=2, space="PSUM") as psc, \
                 tc.tile_pool(name="ppv", bufs=1, space="PSUM") as ppv, \
                 tc.tile_pool(name="po", bufs=1, space="PSUM") as po, \
                 tc.tile_pool(name="expt", bufs=17) as pe, \
                 tc.tile_pool(name="bcp", bufs=2) as pb, \
                 tc.tile_pool(name="dnp", bufs=2) as pd, \
                 tc.tile_pool(name="tmpo", bufs=2) as pt, \
                 tc.tile_pool(name="finp", bufs=2) as pf:
                for ib in range(NIB):
                    for h in range(HPC):
                        pair, hl = h // 2, h % 2
                        pvb = ppv.tile([DK + 1, IBW], F32, tag="pv")
                        exps = []
                        for jt in range(NJT):
                            sc = psc.tile([128, IBW], F32, tag="sc")
                            q_ap = qT[ds(hl * DK, DK), pair]
                            k_ap = kT[ds(hl * DK, DK), pair]
                            for nn in range(IBW // 512):
                                nc.tensor.matmul(
                                    sc[:, ds(nn * 512, 512)],
                                    k_ap[:, ds(jt * 128, 128)],
                                    q_ap[:, ds(ib * IBW + nn * 512, 512)],
                                    start=True,
                                    stop=True,
                                )
                            et = pe.tile([128, IBW], F32, tag="expt")
                            nc.scalar.activation(et[:], sc[:], AF.Exp, scale=SCALE)
                            for nn in range(IBW // 512):
                                nc.tensor.matmul(
                                    pvb[:, ds(nn * 512, 512)],
                                    vsb[:, jt, h],
                                    et[:, ds(nn * 512, 512)],
                                    start=(jt == 0),
                                    stop=(jt == NJT - 1),
                                )
                            exps.append(et)
                        # denominators -> reciprocal -> broadcast to 128 rows
                        rec = pd.tile([1, IBW], F32, tag="dn")
                        nc.vector.reciprocal(rec[:], pvb[ds(DK, 1), :])
                        bc_ps = psc.tile([128, IBW], F32, tag="sc")
                        for nn in range(IBW // 512):
                            nc.tensor.matmul(
                                bc_ps[:, ds(nn * 512, 512)],
                                ones_k1[:],
                                rec[:, ds(nn * 512, 512)],
                                start=True,
                                stop=True,
                            )
                        bc = pb.tile([128, IBW], F32, tag="bc")
                        nc.scalar.copy(bc[:], bc_ps[:])
                        # attention-weight output: normalize in place + store
                        for jt in range(NJT):
                            et = exps[jt]
                            nc.vector.tensor_mul(et[:], et[:], bc[:])
                            nc.sync.dma_start(
                                out=attnT_ap[
                                    h,
                                    ds(jt * 128, 128),
                                    ds(ib * IBW, IBW),
                                ],
                                in_=et[:],
                            )
                        # outT rows for this head, normalized
                        if hl == 0:
                            nc.vector.tensor_mul(
                                outT[ds(0, DK), pair, ds(ib * IBW, IBW)],
                                pvb[ds(0, DK), :],
                                bc[ds(0, DK), :],
                            )
                        else:
                            to = pt.tile([DK, IBW], F32, tag="tmpo")
                            nc.vector.tensor_mul(
                                to[:], pvb[ds(0, DK), :], bc[ds(0, DK), :]
                            )
                            nc.sync.dma_start(
                                out=outT[ds(DK, DK), pair, ds(ib * IBW, IBW)],
                                in_=to[:],
                            )
                    # ---- output projection for this i block ----
                    for iti in range(IBW // 128):
                        it = ib * (IBW // 128) + iti
                        pso = po.tile([128, D], F32, tag="pso")
                        for eh in range(2):
                            for pair in range(2):
                                nc.tensor.matmul(
                                    pso[:, ds(eh * 512, 512)],
                                    outT[:, pair, ds(it * 128, 128)],
                                    wo_sb[:, pair, ds(eh * 512, 512)],
                                    start=(pair == 0),
                                    stop=(pair == 1),
                                )
                        fin = pf.tile([128, D], F32, tag="fin")
                        nc.scalar.copy(fin[:], pso[:])
                        nc.sync.dma_start(
                            out=outp_ap[ds(it * 128, 128), :], in_=fin[:]
                        )

    nc.compile()
    return nc


def _get_nc():
    if "nc" not in _CACHE:
        _CACHE["nc"] = build_nc()
    return _CACHE["nc"]


def make_in_maps(query, key, value, Wq, bq, Wk, bk, Wv, bv, Wo, bo):
    query = np.asarray(query, dtype=np.float32)
    key = np.asarray(key, dtype=np.float32)
    value = np.asarray(value, dtype=np.float32)
    xqT = [np.ascontiguousarray(query[b].T) for b in range(B)]
    xkT = [np.ascontiguousarray(key[b].T) for b in range(B)]
    xvT = [np.ascontiguousarray(value[b].T) for b in range(B)]
    in_maps = []
    for c in range(NCORES):
        b, g = divmod(c, GPB)
        cols = slice(g * DC, (g + 1) * DC)
        in_maps.append(
            {
                "xqT": xqT[b],
                "xkT": xkT[b],
                "xvT": xvT[b],
                "wq": np.ascontiguousarray(np.asarray(Wq, np.float32)[:, cols]),
                "wk": np.ascontiguousarray(np.asarray(Wk, np.float32)[:, cols]),
                "wv": np.ascontiguousarray(np.asarray(Wv, np.float32)[:, cols]),
                "wo": np.ascontiguousarray(np.asarray(Wo, np.float32)[cols, :]),
                "bq": np.asarray(bq, np.float32)[cols].reshape(1, DC),
                "bk": np.asarray(bk, np.float32)[cols].reshape(1, DC),
                "bv": np.asarray(bv, np.float32)[cols].reshape(1, DC),
            }
        )
    return in_maps


def assemble(results, bo):
    attn = np.empty((B, NH, S, S), dtype=np.float32)
    out = np.zeros((B, S, D), dtype=np.float32)
    for c in range(NCORES):
        b, g = divmod(c, GPB)
        r = results[c]
        at = r["attnT"]  # [4, j, i]
        for t in range(HPC):
            attn[b, g * HPC + t] = at[t].T
        out[b] += r["outp"]
    out += np.asarray(bo, np.float32)[None, None, :]
    return out, attn


def kernel(query, key, value, Wq, bq, Wk, bk, Wv, bv, Wo, bo):
    nc = _get_nc()
    in_maps = make_in_maps(query, key, value, Wq, bq, Wk, bk, Wv, bv, Wo, bo)
    res = run_bass_kernel_spmd(nc, in_maps, list(range(NCORES)))
    return assemble(res.results, bo)


# revision 5
# speedup vs baseline: 1.5779x; 1.5779x over previous
"""Multi-head attention (B=2, S=2048, D=1024, H=16) on 8 Trainium2 cores.

Sharding: batch x head-group. Core c handles batch b = c // 4 and heads
[4*(c%4), 4*(c%4)+4). Each core computes its Q/K/V projection slices
(transposed layout), per-head attention with softmax, the attention-weight
output block, and a partial output projection over its 256 channels.
Host sums the 4 partial output projections per batch element and adds bo.

All device compute is fp32. Scores are built transposed ([j, i] = [key,
query]) so the PV matmul can consume them directly; a ones column appended
to V yields the softmax denominators from the same matmul. exp() normalizer
is applied with a PE-broadcast reciprocal row. The attention output is
written [h, j, i] and transposed on the host.
"""

import numpy as np

import concourse.bass as bass
import concourse.tile as tile
from concourse import bacc, mybir
from concourse.bass import ds
from concourse.bass_utils import run_bass_kernel_spmd

F32 = mybir.dt.float32
F32R = mybir.dt.float32r
AF = mybir.ActivationFunctionType


def _r(ap):
    return ap.bitcast(F32R)

B = 2
S = 2048
D = 1024
NH = 16
DK = 64
NCORES = 8
GPB = 4                # cores (head-groups) per batch element
HPC = NH // GPB        # heads per core = 4
DC = HPC * DK          # channels per core = 256
KC = D // 128          # contraction chunks = 8
NJT = S // 128         # j tiles = 16
IBW = 1024             # i block width
NIB = S // IBW         # i blocks = 2
NST = S // 128         # s tiles for V projection = 16
SCALE = float(1.0 / np.sqrt(np.float32(DK)))

_CACHE = {}


def build_nc():
    nc = bacc.Bacc("TRN2", target_bir_lowering=False, debug=False)

    xq = nc.declare_dram_parameter("xqT", [D, S], F32, isOutput=False)
    xk = nc.declare_dram_parameter("xkT", [D, S], F32, isOutput=False)
    xv = nc.declare_dram_parameter("xvT", [D, S], F32, isOutput=False)
    wq = nc.declare_dram_parameter("wq", [D, DC], F32, isOutput=False)
    wk = nc.declare_dram_parameter("wk", [D, DC], F32, isOutput=False)
    wv = nc.declare_dram_parameter("wv", [D, DC], F32, isOutput=False)
    wo = nc.declare_dram_parameter("wo", [DC, D], F32, isOutput=False)
    bq = nc.declare_dram_parameter("bq", [1, DC], F32, isOutput=False)
    bk = nc.declare_dram_parameter("bk", [1, DC], F32, isOutput=False)
    bv = nc.declare_dram_parameter("bv", [1, DC], F32, isOutput=False)
    attnT = nc.declare_dram_parameter("attnT", [HPC, S, S], F32, isOutput=True)
    outp = nc.declare_dram_parameter("outp", [S, D], F32, isOutput=True)

    # chunked DRAM views: row k = c*128 + p  ->  [p, c, ...]
    xq_c = xq.ap().rearrange("(c p) s -> p c s", p=128)
    xk_c = xk.ap().rearrange("(c p) s -> p c s", p=128)
    xv_c = xv.ap().rearrange("(c p) s -> p c s", p=128)
    wq_c = wq.ap().rearrange("(c p) n -> p c n", p=128)
    wk_c = wk.ap().rearrange("(c p) n -> p c n", p=128)
    wv_c = wv.ap().rearrange("(c p) n -> p c n", p=128)
    wo_c = wo.ap().rearrange("(c p) n -> p c n", p=128)
    attnT_ap = attnT.ap()
    outp_ap = outp.ap()

    with tile.TileContext(nc) as tc:
        with tc.tile_pool(name="persist", bufs=1) as pp:
            qT = pp.tile([128, 2, S], F32, tag="qT")     # [pair-chan, pair, s]
            kT = pp.tile([128, 2, S], F32, tag="kT")
            vsb = pp.tile([128, NST, HPC, DK + 1], F32, tag="vsb")
            wo_sb = pp.tile([128, 2, D], F32, tag="wo")
            outT = pp.tile([128, 2, S], F32, tag="outT")  # [pair-chan, pair, i]
            ones_k1 = pp.tile([1, 128], F32, tag="ones_k1")

            nc.any.memset(ones_k1[:], 1.0)
            nc.any.memset(vsb[:, :, :, DK], 1.0)  # denominator ones column
            nc.sync.dma_start(out=wo_sb[:], in_=wo_c)

            # ---------------- Phase 1: projections ----------------
            with tc.tile_pool(name="ph1", bufs=1) as wp:
                wq_sb = wp.tile([128, KC, DC], F32, tag="wq")
                wk_sb = wp.tile([128, KC, DC], F32, tag="wk")
                wv_sb = wp.tile([128, KC, DC], F32, tag="wv")
                bq_sb = wp.tile([1, DC], F32, tag="bq")
                bk_sb = wp.tile([1, DC], F32, tag="bk")
                bv_sb = wp.tile([1, DC], F32, tag="bv")
                ones_row = wp.tile([1, 512], F32, tag="ones_row")
                nc.sync.dma_start(out=wq_sb[:], in_=wq_c)
                nc.sync.dma_start(out=wk_sb[:], in_=wk_c)
                nc.sync.dma_start(out=wv_sb[:], in_=wv_c)
                nc.sync.dma_start(out=bq_sb[:], in_=bq.ap())
                nc.sync.dma_start(out=bk_sb[:], in_=bk.ap())
                nc.sync.dma_start(out=bv_sb[:], in_=bv.ap())
                nc.any.memset(ones_row[:], 1.0)

                # QT / KT: out[d, s] = sum_k W[k, d] * xT[k, s]  (+ bias)
                with tc.tile_pool(name="ph1qk", bufs=2, space="PSUM") as pqk, \
                     tc.tile_pool(name="ph1x", bufs=3) as px:
                    for (x_c, w_sb, b_sb, dst) in (
                        (xq_c, wq_sb, bq_sb, qT),
                        (xk_c, wk_sb, bk_sb, kT),
                    ):
                        ps = [
                            pqk.tile([128, S], F32, tag="pqk", name=f"pqk{i}")
                            for i in range(2)
                        ]
                        for kc in range(KC):
                            xc = px.tile([128, S], F32, tag="xc")
                            nc.sync.dma_start(out=xc[:], in_=x_c[:, kc])
                            for pair in range(2):
                                lhsT = w_sb[:, kc, ds(pair * 128, 128)]
                                for n in range(S // 512):
                                    nc.tensor.matmul(
                                        ps[pair][:, ds(n * 512, 512)],
                                        lhsT,
                                        xc[:, ds(n * 512, 512)],
                                        start=(kc == 0),
                                        stop=False,
                                    )
                        # bias row: out[d, s] += b[d] * 1
                        for pair in range(2):
                            lhsT_b = b_sb[:, ds(pair * 128, 128)]
                            for n in range(S // 512):
                                nc.tensor.matmul(
                                    ps[pair][:, ds(n * 512, 512)],
                                    lhsT_b,
                                    ones_row[:],
                                    start=False,
                                    stop=(n == S // 512 - 1),
                                )
                            nc.scalar.copy(dst[:, pair], ps[pair][:])

                # V: out[s, d] = sum_k xvT[k, s] * W[k, d]  (+ bias)
                with tc.tile_pool(name="ph1v", bufs=4, space="PSUM") as pv, \
                     tc.tile_pool(name="ph1xv", bufs=3) as pxv:
                    for stg in range(4):  # groups of 4 s-tiles
                        xvs = []
                        for kc in range(KC):
                            xt = pxv.tile([128, 512], F32, tag="xv")
                            nc.sync.dma_start(
                                out=xt[:], in_=xv_c[:, kc, ds(stg * 512, 512)]
                            )
                            xvs.append(xt)
                        for sti in range(4):
                            st = stg * 4 + sti
                            psv = pv.tile([128, DC], F32, tag="psv")
                            for kc in range(KC):
                                nc.tensor.matmul(
                                    psv[:],
                                    _r(xvs[kc][:, ds(sti * 128, 128)]),
                                    _r(wv_sb[:, kc]),
                                    start=(kc == 0),
                                    stop=False,
                                )
                            nc.tensor.matmul(
                                psv[:], _r(ones_k1[:]), _r(bv_sb[:]),
                                start=False, stop=True,
                            )
                            nc.scalar.copy(
                                vsb[:, st, :, 0:DK],
                                psv[:].rearrange("p (h d) -> p h d", h=HPC),
                            )

            # ---------------- Phase 2: attention ----------------
            with tc.tile_pool(name="psc", bufs=2, space="PSUM") as psc, \
                 tc.tile_pool(name="ppv", bufs=1, space="PSUM") as ppv, \
                 tc.tile_pool(name="po", bufs=1, space="PSUM") as po, \
                 tc.tile_pool(name="expt", bufs=17) as pe, \
                 tc.tile_pool(name="bcp", bufs=2) as pb, \
                 tc.tile_pool(name="dnp", bufs=2) as pd, \
                 tc.tile_pool(name="tmpo", bufs=2) as pt, \
                 tc.tile_pool(name="finp", bufs=2) as pf:
                for ib in range(NIB):
                    for h in range(HPC):
                        pair, hl = h // 2, h % 2
                        pvb = ppv.tile([DK + 1, IBW], F32, tag="pv")
                        exps = []
                        for jt in range(NJT):
                            sc = psc.tile([128, IBW], F32, tag="sc")
                            q_ap = qT[ds(hl * DK, DK), pair]
                            k_ap = kT[ds(hl * DK, DK), pair]
                            for nn in range(IBW // 512):
                                nc.tensor.matmul(
                                    sc[:, ds(nn * 512, 512)],
                                    _r(k_ap[:, ds(jt * 128, 128)]),
                                    _r(q_ap[:, ds(ib * IBW + nn * 512, 512)]),
                                    start=True,
                                    stop=True,
                                )
                            et = pe.tile([128, IBW], F32, tag="expt")
                            nc.scalar.activation(et[:], sc[:], AF.Exp, scale=SCALE)
                            for nn in range(IBW // 512):
                                nc.tensor.matmul(
                                    pvb[:, ds(nn * 512, 512)],
                                    _r(vsb[:, jt, h]),
                                    _r(et[:, ds(nn * 512, 512)]),
                                    start=(jt == 0),
                                    stop=(jt == NJT - 1),
                                )
                            exps.append(et)
                        # denominators -> reciprocal -> broadcast to 128 rows
                        rec = pd.tile([1, IBW], F32, tag="dn")
                        nc.vector.reciprocal(rec[:], pvb[ds(DK, 1), :])
                        bc_ps = psc.tile([128, IBW], F32, tag="sc")
                        for nn in range(IBW // 512):
                            nc.tensor.matmul(
                                bc_ps[:, ds(nn * 512, 512)],
                                _r(ones_k1[:]),
                                _r(rec[:, ds(nn * 512, 512)]),
                                start=True,
                                stop=True,
                            )
                        bc = pb.tile([128, IBW], F32, tag="bc")
                        nc.scalar.copy(bc[:], bc_ps[:])
                        # attention-weight output: normalize in place + store
                        for jt in range(NJT):
                            et = exps[jt]
                            nc.vector.tensor_mul(et[:], et[:], bc[:])
                            nc.sync.dma_start(
                                out=attnT_ap[
                                    h,
                                    ds(jt * 128, 128),
                                    ds(ib * IBW, IBW),
                                ],
                                in_=et[:],
                            )
                        # outT rows for this head, normalized
                        if hl == 0:
                            nc.vector.tensor_mul(
                                outT[ds(0, DK), pair, ds(ib * IBW, IBW)],
                                pvb[ds(0, DK), :],
                                bc[ds(0, DK), :],
                            )
                        else:
                            to = pt.tile([DK, IBW], F32, tag="tmpo")
                            nc.vector.tensor_mul(
                                to[:], pvb[ds(0, DK), :], bc[ds(0, DK), :]
                            )
                            nc.sync.dma_start(
                                out=outT[ds(DK, DK), pair, ds(ib * IBW, IBW)],
                                in_=to[:],
                            )
                    # ---- output projection for this i block ----
                    for iti in range(IBW // 128):
                        it = ib * (IBW // 128) + iti
                        pso = po.tile([128, D], F32, tag="pso")
                        for eh in range(2):
                            for pair in range(2):
                                nc.tensor.matmul(
                                    pso[:, ds(eh * 512, 512)],
                                    _r(outT[:, pair, ds(it * 128, 128)]),
                                    _r(wo_sb[:, pair, ds(eh * 512, 512)]),
                                    start=(pair == 0),
                                    stop=(pair == 1),
                                )
                        fin = pf.tile([128, D], F32, tag="fin")
                        nc.scalar.copy(fin[:], pso[:])
                        nc.sync.dma_start(
                            out=outp_ap[ds(it * 128, 128), :], in_=fin[:]
                        )

    nc.compile()
    return nc


def _get_nc():
    if "nc" not in _CACHE:
        _CACHE["nc"] = build_nc()
    return _CACHE["nc"]


def make_in_maps(query, key, value, Wq, bq, Wk, bk, Wv, bv, Wo, bo):
    query = np.asarray(query, dtype=np.float32)
    key = np.asarray(key, dtype=np.float32)
    value = np.asarray(value, dtype=np.float32)
    xqT = [np.ascontiguousarray(query[b].T) for b in range(B)]
    xkT = [np.ascontiguousarray(key[b].T) for b in range(B)]
    xvT = [np.ascontiguousarray(value[b].T) for b in range(B)]
    in_maps = []
    for c in range(NCORES):
        b, g = divmod(c, GPB)
        cols = slice(g * DC, (g + 1) * DC)
        in_maps.append(
            {
                "xqT": xqT[b],
                "xkT": xkT[b],
                "xvT": xvT[b],
                "wq": np.ascontiguousarray(np.asarray(Wq, np.float32)[:, cols]),
                "wk": np.ascontiguousarray(np.asarray(Wk, np.float32)[:, cols]),
                "wv": np.ascontiguousarray(np.asarray(Wv, np.float32)[:, cols]),
                "wo": np.ascontiguousarray(np.asarray(Wo, np.float32)[cols, :]),
                "bq": np.asarray(bq, np.float32)[cols].reshape(1, DC),
                "bk": np.asarray(bk, np.float32)[cols].reshape(1, DC),
                "bv": np.asarray(bv, np.float32)[cols].reshape(1, DC),
            }
        )
    return in_maps


def assemble(results, bo):
    attn = np.empty((B, NH, S, S), dtype=np.float32)
    out = np.zeros((B, S, D), dtype=np.float32)
    for c in range(NCORES):
        b, g = divmod(c, GPB)
        r = results[c]
        at = r["attnT"]  # [4, j, i]
        for t in range(HPC):
            attn[b, g * HPC + t] = at[t].T
        out[b] += r["outp"]
    out += np.asarray(bo, np.float32)[None, None, :]
    return out, attn


def kernel(query, key, value, Wq, bq, Wk, bk, Wv, bv, Wo, bo):
    nc = _get_nc()
    in_maps = make_in_maps(query, key, value, Wq, bq, Wk, bk, Wv, bv, Wo, bo)
    res = run_bass_kernel_spmd(nc, in_maps, list(range(NCORES)))
    return assemble(res.results, bo)


# revision 6
# speedup vs baseline: 1.8337x; 1.1622x over previous
"""Multi-head attention (B=2, S=2048, D=1024, H=16) on 8 Trainium2 cores.

Sharding: batch x head-group. Core c handles batch b = c // 4 and heads
[4*(c%4), 4*(c%4)+4). Each core computes its Q/K/V projection slices
(transposed layout), per-head attention with softmax, the attention-weight
output block, and a partial output projection over its 256 channels.
Host sums the 4 partial output projections per batch element and adds bo.

All device compute is fp32. Scores are built transposed ([j, i] = [key,
query]) so the PV matmul can consume them directly; a ones column appended
to V yields the softmax denominators from the same matmul. exp() normalizer
is applied with a PE-broadcast reciprocal row. The attention output is
written [h, j, i] and transposed on the host.
"""

import numpy as np

import concourse.bass as bass
import concourse.tile as tile
from concourse import bacc, mybir
from concourse.bass import ds
from concourse.bass_utils import run_bass_kernel_spmd

F32 = mybir.dt.float32
F32R = mybir.dt.float32r
AF = mybir.ActivationFunctionType


def _r(ap):
    return ap.bitcast(F32R)

B = 2
S = 2048
D = 1024
NH = 16
DK = 64
NCORES = 8
GPB = 4                # cores (head-groups) per batch element
HPC = NH // GPB        # heads per core = 4
DC = HPC * DK          # channels per core = 256
KC = D // 128          # contraction chunks = 8
NJT = S // 128         # j tiles = 16
IBW = 1024             # i block width
NIB = S // IBW         # i blocks = 2
NST = S // 128         # s tiles for V projection = 16
SCALE = float(1.0 / np.sqrt(np.float32(DK)))

_CACHE = {}


def build_nc():
    nc = bacc.Bacc("TRN2", target_bir_lowering=False, debug=False)

    xq = nc.declare_dram_parameter("xqT", [D, S], F32, isOutput=False)
    xk = nc.declare_dram_parameter("xkT", [D, S], F32, isOutput=False)
    xv = nc.declare_dram_parameter("xvT", [D, S], F32, isOutput=False)
    wq = nc.declare_dram_parameter("wq", [D, DC], F32, isOutput=False)
    wk = nc.declare_dram_parameter("wk", [D, DC], F32, isOutput=False)
    wv = nc.declare_dram_parameter("wv", [D, DC], F32, isOutput=False)
    wo = nc.declare_dram_parameter("wo", [DC, D], F32, isOutput=False)
    bq = nc.declare_dram_parameter("bq", [1, DC], F32, isOutput=False)
    bk = nc.declare_dram_parameter("bk", [1, DC], F32, isOutput=False)
    bv = nc.declare_dram_parameter("bv", [1, DC], F32, isOutput=False)
    attnT = nc.declare_dram_parameter("attnT", [HPC, S, S], F32, isOutput=True)
    outp = nc.declare_dram_parameter("outp", [S, D], F32, isOutput=True)

    # chunked DRAM views: row k = c*128 + p  ->  [p, c, ...]
    xq_c = xq.ap().rearrange("(c p) s -> p c s", p=128)
    xk_c = xk.ap().rearrange("(c p) s -> p c s", p=128)
    xv_c = xv.ap().rearrange("(c p) s -> p c s", p=128)
    wq_c = wq.ap().rearrange("(c p) n -> p c n", p=128)
    wk_c = wk.ap().rearrange("(c p) n -> p c n", p=128)
    wv_c = wv.ap().rearrange("(c p) n -> p c n", p=128)
    wo_c = wo.ap().rearrange("(c p) n -> p c n", p=128)
    attnT_ap = attnT.ap()
    outp_ap = outp.ap()

    with tile.TileContext(nc) as tc:
        with tc.tile_pool(name="persist", bufs=1) as pp:
            qT = pp.tile([128, 2, S], F32, tag="qT")     # [pair-chan, pair, s]
            kT = pp.tile([128, 2, S], F32, tag="kT")
            vsb = pp.tile([128, NST, HPC, DK + 1], F32, tag="vsb")
            wo_sb = pp.tile([128, 2, D], F32, tag="wo")
            outT = pp.tile([128, 2, S], F32, tag="outT")  # [pair-chan, pair, i]
            ones_k1 = pp.tile([1, 128], F32, tag="ones_k1")

            nc.any.memset(ones_k1[:], 1.0)
            nc.any.memset(vsb[:, :, :, DK], 1.0)  # denominator ones column
            nc.sync.dma_start(out=wo_sb[:], in_=wo_c)

            # ---------------- Phase 1: projections ----------------
            with tc.tile_pool(name="ph1", bufs=1) as wp:
                wq_sb = wp.tile([128, KC, DC], F32, tag="wq")
                wk_sb = wp.tile([128, KC, DC], F32, tag="wk")
                wv_sb = wp.tile([128, KC, DC], F32, tag="wv")
                bq_sb = wp.tile([1, DC], F32, tag="bq")
                bk_sb = wp.tile([1, DC], F32, tag="bk")
                bv_sb = wp.tile([1, DC], F32, tag="bv")
                ones_row = wp.tile([1, 512], F32, tag="ones_row")
                nc.sync.dma_start(out=wq_sb[:], in_=wq_c)
                nc.sync.dma_start(out=wk_sb[:], in_=wk_c)
                nc.sync.dma_start(out=wv_sb[:], in_=wv_c)
                nc.sync.dma_start(out=bq_sb[:], in_=bq.ap())
                nc.sync.dma_start(out=bk_sb[:], in_=bk.ap())
                nc.sync.dma_start(out=bv_sb[:], in_=bv.ap())
                nc.any.memset(ones_row[:], 1.0)

                # QT / KT: out[d, s] = sum_k W[k, d] * xT[k, s]  (+ bias)
                with tc.tile_pool(name="ph1qk", bufs=2, space="PSUM") as pqk, \
                     tc.tile_pool(name="ph1x", bufs=3) as px:
                    for (x_c, w_sb, b_sb, dst) in (
                        (xq_c, wq_sb, bq_sb, qT),
                        (xk_c, wk_sb, bk_sb, kT),
                    ):
                        ps = [
                            pqk.tile([128, S], F32, tag="pqk", name=f"pqk{i}")
                            for i in range(2)
                        ]
                        for kc in range(KC):
                            xc = px.tile([128, S], F32, tag="xc")
                            nc.sync.dma_start(out=xc[:], in_=x_c[:, kc])
                            for pair in range(2):
                                lhsT = w_sb[:, kc, ds(pair * 128, 128)]
                                for n in range(S // 512):
                                    nc.tensor.matmul(
                                        ps[pair][:, ds(n * 512, 512)],
                                        _r(lhsT),
                                        _r(xc[:, ds(n * 512, 512)]),
                                        start=(kc == 0),
                                        stop=False,
                                    )
                        # bias row: out[d, s] += b[d] * 1
                        for pair in range(2):
                            lhsT_b = b_sb[:, ds(pair * 128, 128)]
                            for n in range(S // 512):
                                nc.tensor.matmul(
                                    ps[pair][:, ds(n * 512, 512)],
                                    lhsT_b,
                                    ones_row[:],
                                    start=False,
                                    stop=(n == S // 512 - 1),
                                )
                            nc.scalar.copy(dst[:, pair], ps[pair][:])

                # V: out[s, d] = sum_k xvT[k, s] * W[k, d]  (+ bias)
                with tc.tile_pool(name="ph1v", bufs=4, space="PSUM") as pv, \
                     tc.tile_pool(name="ph1xv", bufs=3) as pxv:
                    for stg in range(4):  # groups of 4 s-tiles
                        xvs = []
                        for kc in range(KC):
                            xt = pxv.tile([128, 512], F32, tag="xv")
                            nc.sync.dma_start(
                                out=xt[:], in_=xv_c[:, kc, ds(stg * 512, 512)]
                            )
                            xvs.append(xt)
                        for sti in range(4):
                            st = stg * 4 + sti
                            psv = pv.tile([128, DC], F32, tag="psv")
                            for kc in range(KC):
                                nc.tensor.matmul(
                                    psv[:],
                                    _r(xvs[kc][:, ds(sti * 128, 128)]),
                                    _r(wv_sb[:, kc]),
                                    start=(kc == 0),
                                    stop=False,
                                )
                            nc.tensor.matmul(
                                psv[:], _r(ones_k1[:]), _r(bv_sb[:]),
                                start=False, stop=True,
                            )
                            nc.scalar.copy(
                                vsb[:, st, :, 0:DK],
                                psv[:].rearrange("p (h d) -> p h d", h=HPC),
                            )

            # ---------------- Phase 2: attention ----------------
            with tc.tile_pool(name="psc", bufs=2, space="PSUM") as psc, \
                 tc.tile_pool(name="ppv", bufs=2, space="PSUM") as ppv, \
                 tc.tile_pool(name="expt", bufs=17) as pe, \
                 tc.tile_pool(name="bcp", bufs=2) as pb, \
                 tc.tile_pool(name="dnp", bufs=2) as pd, \
                 tc.tile_pool(name="tmpo", bufs=2) as pt, \
                 tc.tile_pool(name="finp", bufs=2) as pf:
                for ib in range(NIB):
                    for h in range(HPC):
                        pair, hl = h // 2, h % 2
                        pvb = ppv.tile([DK + 1, IBW], F32, tag="pv")
                        exps = []
                        for jt in range(NJT):
                            sc = psc.tile([128, IBW], F32, tag="sc")
                            q_ap = qT[ds(hl * DK, DK), pair]
                            k_ap = kT[ds(hl * DK, DK), pair]
                            for nn in range(IBW // 512):
                                nc.tensor.matmul(
                                    sc[:, ds(nn * 512, 512)],
                                    _r(k_ap[:, ds(jt * 128, 128)]),
                                    _r(q_ap[:, ds(ib * IBW + nn * 512, 512)]),
                                    start=True,
                                    stop=True,
                                )
                            et = pe.tile([128, IBW], F32, tag="expt")
                            nc.scalar.activation(et[:], sc[:], AF.Exp, scale=SCALE)
                            for nn in range(IBW // 512):
                                nc.tensor.matmul(
                                    pvb[:, ds(nn * 512, 512)],
                                    _r(vsb[:, jt, h]),
                                    _r(et[:, ds(nn * 512, 512)]),
                                    start=(jt == 0),
                                    stop=(jt == NJT - 1),
                                )
                            exps.append(et)
                        # denominators -> reciprocal -> broadcast to 128 rows
                        rec = pd.tile([1, IBW], F32, tag="dn")
                        nc.vector.reciprocal(rec[:], pvb[ds(DK, 1), :])
                        bc_ps = psc.tile([128, IBW], F32, tag="sc")
                        for nn in range(IBW // 512):
                            nc.tensor.matmul(
                                bc_ps[:, ds(nn * 512, 512)],
                                _r(ones_k1[:]),
                                _r(rec[:, ds(nn * 512, 512)]),
                                start=True,
                                stop=True,
                            )
                        bc = pb.tile([128, IBW], F32, tag="bc")
                        nc.scalar.copy(bc[:], bc_ps[:])
                        # attention-weight output: normalize in place + store
                        for jt in range(NJT):
                            et = exps[jt]
                            if jt % 3 == 2:
                                nc.gpsimd.tensor_mul(et[:], et[:], bc[:])
                            else:
                                nc.vector.tensor_mul(et[:], et[:], bc[:])
                            nc.sync.dma_start(
                                out=attnT_ap[
                                    h,
                                    ds(jt * 128, 128),
                                    ds(ib * IBW, IBW),
                                ],
                                in_=et[:],
                            )
                        # outT rows for this head, normalized
                        if hl == 0:
                            nc.vector.tensor_mul(
                                outT[ds(0, DK), pair, ds(ib * IBW, IBW)],
                                pvb[ds(0, DK), :],
                                bc[ds(0, DK), :],
                            )
                        else:
                            to = pt.tile([DK, IBW], F32, tag="tmpo")
                            nc.vector.tensor_mul(
                                to[:], pvb[ds(0, DK), :], bc[ds(0, DK), :]
                            )
                            nc.sync.dma_start(
                                out=outT[ds(DK, DK), pair, ds(ib * IBW, IBW)],
                                in_=to[:],
                            )
                    # ---- output projection for this i block ----
                    for iti in range(IBW // 128):
                        it = ib * (IBW // 128) + iti
                        pso = psc.tile([128, D], F32, tag="sc", name="pso")
                        for eh in range(2):
                            for pair in range(2):
                                nc.tensor.matmul(
                                    pso[:, ds(eh * 512, 512)],
                                    _r(outT[:, pair, ds(it * 128, 128)]),
                                    _r(wo_sb[:, pair, ds(eh * 512, 512)]),
                                    start=(pair == 0),
                                    stop=(pair == 1),
                                )
                        fin = pf.tile([128, D], F32, tag="fin")
                        nc.scalar.copy(fin[:], pso[:])
                        nc.sync.dma_start(
                            out=outp_ap[ds(it * 128, 128), :], in_=fin[:]
                        )

    nc.compile()
    return nc


def _get_nc():
    if "nc" not in _CACHE:
        _CACHE["nc"] = build_nc()
    return _CACHE["nc"]


def make_in_maps(query, key, value, Wq, bq, Wk, bk, Wv, bv, Wo, bo):
    query = np.asarray(query, dtype=np.float32)
    key = np.asarray(key, dtype=np.float32)
    value = np.asarray(value, dtype=np.float32)
    xqT = [np.ascontiguousarray(query[b].T) for b in range(B)]
    xkT = [np.ascontiguousarray(key[b].T) for b in range(B)]
    xvT = [np.ascontiguousarray(value[b].T) for b in range(B)]
    in_maps = []
    for c in range(NCORES):
        b, g = divmod(c, GPB)
        cols = slice(g * DC, (g + 1) * DC)
        in_maps.append(
            {
                "xqT": xqT[b],
                "xkT": xkT[b],
                "xvT": xvT[b],
                "wq": np.ascontiguousarray(np.asarray(Wq, np.float32)[:, cols]),
                "wk": np.ascontiguousarray(np.asarray(Wk, np.float32)[:, cols]),
                "wv": np.ascontiguousarray(np.asarray(Wv, np.float32)[:, cols]),
                "wo": np.ascontiguousarray(np.asarray(Wo, np.float32)[cols, :]),
                "bq": np.asarray(bq, np.float32)[cols].reshape(1, DC),
                "bk": np.asarray(bk, np.float32)[cols].reshape(1, DC),
                "bv": np.asarray(bv, np.float32)[cols].reshape(1, DC),
            }
        )
    return in_maps


def assemble(results, bo):
    attn = np.empty((B, NH, S, S), dtype=np.float32)
    out = np.zeros((B, S, D), dtype=np.float32)
    for c in range(NCORES):
        b, g = divmod(c, GPB)
        r = results[c]
        at = r["attnT"]  # [4, j, i]
        for t in range(HPC):
            attn[b, g * HPC + t] = at[t].T
        out[b] += r["outp"]
    out += np.asarray(bo, np.float32)[None, None, :]
    return out, attn


def kernel(query, key, value, Wq, bq, Wk, bk, Wv, bv, Wo, bo):
    nc = _get_nc()
    in_maps = make_in_maps(query, key, value, Wq, bq, Wk, bk, Wv, bv, Wo, bo)
    res = run_bass_kernel_spmd(nc, in_maps, list(range(NCORES)))
    return assemble(res.results, bo)
